# revision 17
# baseline (speedup 1.0000x reference)
"""Trainium2 Bass kernel: ViT attention block with 2D RoPE (croco-style).

Full inputs -> full outputs. Sharding: data-parallel over batch, one batch
element per NeuronCore (B=8 across 8 cores), no collectives.

v5a: PE-paced deep pipeline.
  - Lag-2 attn@v: attn@v for (t,c0) consumes the exp tile from 2 iterations
    earlier so the PE never waits on the activation engine.
  - exp split ACT 13 / DVE 3 via a Schraudolph bit-trick on DVE
    (y = round(a*x+b) as int16, bitcast to bf16).
  - 4-phase oacc ([128,512] per (q-half, head)) + [128,512] aux psum ring:
    eps 2x2 + aux 2x1 + oacc 2x1 = 8 banks.
  - fixE (q . k_cls column) folded to prologue: u_h = Wq_h^T k_cls on host,
    fixE = U^T x as one matmul set + one exp.
  - cls-query scores via w_h = Wk_h^T q_cls on host: psT = x^T W at tail,
    no PE transposes; q_cls.k_cls from host (pcls).
  - Normalization inlined: den rows copied out per phase, reciprocal runs
    at [128,16] via DMA partition-reshape (recip is ~6.5ns/free-elem),
    sel-matmul broadcast + mul during next pair.
  - proj bias via broadcast tensor_add; rope-swap/reshape DMAs on the
    gpsimd queue (cheap dispatch). GPSIMD does SBUF-only work (no PSUM).
"""

import numpy as np
import ml_dtypes

import concourse.bass as bass
import concourse.mybir as mybir
import concourse.tile as tile
from concourse import bacc
from concourse.bass_utils import run_bass_kernel_spmd

F32 = mybir.dt.float32
BF16 = mybir.dt.bfloat16
I16 = mybir.dt.int16
EXP = mybir.ActivationFunctionType.Exp
MULT = mybir.AluOpType.mult
ADD = mybir.AluOpType.add

DIM = 768
H = 12
HD = 64
N = 1025
NP = 1024
NC = 8
SCALE = HD ** -0.5

# Schraudolph fast-exp constants for bf16 bit targets:
#   bits = round(EXPA * raw_score + EXPB); bits viewed as bf16 ~= exp(score*SCALE)
EXPA = SCALE * 128.0 * 1.4426950408889634
EXPB = 127.0 * 128.0 - 7.422

EXP_DVE = frozenset({2, 7, 10, 13})

_CACHE = {}


def _build_body(tc):
    nc = tc.nc
    import contextlib
    ctx = contextlib.ExitStack()

    # ---- DRAM inputs (all host-prepped) ----
    xTd = nc.dram_tensor("xT", [DIM, NP], BF16, kind="ExternalInput")
    wqd = nc.dram_tensor("wq", [DIM, 3 * DIM], BF16, kind="ExternalInput")
    wpd = nc.dram_tensor("wp", [DIM, DIM], BF16, kind="ExternalInput")
    ctd = nc.dram_tensor("ct", [128, NP], BF16, kind="ExternalInput")
    sstd = nc.dram_tensor("sst", [128, NP], BF16, kind="ExternalInput")
    vclsd = nc.dram_tensor("vcls", [1, 12 * 128], BF16, kind="ExternalInput")
    ufixd = nc.dram_tensor("ufix", [128, 72], BF16, kind="ExternalInput")
    wfixd = nc.dram_tensor("wfix", [128, 72], BF16, kind="ExternalInput")
    pclsd = nc.dram_tensor("pcls", [1, 12], BF16, kind="ExternalInput")
    seld = nc.dram_tensor("sel", [12, DIM], BF16, kind="ExternalInput")
    bpd = nc.dram_tensor("bp", [1, DIM], BF16, kind="ExternalInput")
    out = nc.dram_tensor("out", [N, DIM], F32, kind="ExternalOutput")

    main = ctx.enter_context(tc.tile_pool(name="main", bufs=1))

    xTt = [main.tile([128, NP], BF16, name=f"xT{j}") for j in range(6)]
    wqt = [main.tile([128, 3 * DIM], BF16, name=f"wq{j}") for j in range(6)]
    wpt = [main.tile([128, DIM], BF16, name=f"wp{j}") for j in range(6)]
    ctt = main.tile([128, NP], BF16, name="ctt")
    sstt = main.tile([128, NP], BF16, name="sstt")
    vclst = main.tile([1, 12 * 128], BF16, name="vclst")
    ufixt = main.tile([128, 72], BF16, name="ufixt")
    wfixt = main.tile([128, 72], BF16, name="wfixt")
    pclst = main.tile([1, 12], BF16, name="pclst")
    selt = main.tile([12, DIM], BF16, name="selt")
    bpt = main.tile([1, DIM], BF16, name="bpt")
    biasf = main.tile([128, DIM], BF16, name="biasf")

    qT = [main.tile([128, NP], BF16, name=f"qT{j}") for j in range(6)]
    kT = [main.tile([128, NP], BF16, name=f"kT{j}") for j in range(6)]
    qTr = [main.tile([128, NP], BF16, name=f"qTr{j}") for j in range(6)]
    kTr = [main.tile([128, NP], BF16, name=f"kTr{j}") for j in range(6)]
    vA = [main.tile([128, 12, 128], BF16, name=f"vA{t}") for t in range(8)]
    oT = [main.tile([128, N], BF16, name=f"oT{j}") for j in range(6)]
    denrcb = main.tile([12, NP], BF16, name="denrcb")
    et0A = main.tile([12, NP], BF16, name="et0A")
    et0w = main.tile([1, 12 * NP], BF16, name="et0w")
    pTt = [main.tile([128, 12], BF16, name=f"pT{t}") for t in range(8)]

    # ---- input DMAs: hot set on sync queue (priority order) ----
    for kc in range(6):
        nc.sync.dma_start(xTt[kc][:], xTd[kc * 128:(kc + 1) * 128, :])
        nc.sync.dma_start(wqt[kc][:, 0:2 * DIM],
                          wqd[kc * 128:(kc + 1) * 128, 0:2 * DIM])
    for kc in range(6):
        nc.sync.dma_start(wqt[kc][:, 2 * DIM:3 * DIM],
                          wqd[kc * 128:(kc + 1) * 128, 2 * DIM:3 * DIM])
    nc.sync.dma_start(ctt[:], ctd[:])
    nc.sync.dma_start(sstt[:], sstd[:])
    # cold set on gpsimd queue (cheap dispatch)
    nc.gpsimd.dma_start(ufixt[:], ufixd[:])
    nc.gpsimd.dma_start(vclst[:], vclsd[:])
    nc.gpsimd.dma_start(bpt[:], bpd[:])
    nc.gpsimd.dma_start(selt[:], seld[:])
    nc.gpsimd.dma_start(wfixt[:], wfixd[:])
    nc.gpsimd.dma_start(pclst[:], pclsd[:])
    for kc in range(6):
        nc.gpsimd.dma_start(wpt[kc][:], wpd[kc * 128:(kc + 1) * 128, :])

    for t in range(8):
        nc.gpsimd.memset(vA[t][:, :, 64:128], 1.0)
    nc.gpsimd.partition_broadcast(biasf[:], bpt[0:1, :])

    psB_cm = tc.tile_pool(name="psB", bufs=1, space="PSUM")
    psB = psB_cm.__enter__()
    sbB_cm = tc.tile_pool(name="sbB", bufs=1)
    sbB = sbB_cm.__enter__()

    # ---------- emission helpers ----------
    def emit_qk_half(j, sec, half):
        # sec: 0 for q, DIM for k; half: token cols half*512
        ps = psB.tile([128, 512], F32, name="qkps", tag="aux", bufs=2)
        c0 = half * 512
        for kc in range(6):
            nc.tensor.matmul(
                ps[:, :],
                wqt[kc][:, sec + j * 128:sec + (j + 1) * 128],
                xTt[kc][:, c0:c0 + 512],
                start=(kc == 0), stop=(kc == 5))
        dst = qT[j] if sec == 0 else kT[j]
        nc.scalar.copy(dst[:, c0:c0 + 512], ps[:, :])

    def emit_rope(j, which):
        src, dst = (qT[j], qTr[j]) if which == 0 else (kT[j], kTr[j])
        qp = sbB.tile([128, NP], BF16, name="qp", tag="qp", bufs=2)
        for b32 in range(4):
            p0 = b32 * 32
            nc.gpsimd.dma_start(qp[p0:p0 + 16, :], src[p0 + 16:p0 + 32, :])
            nc.gpsimd.dma_start(qp[p0 + 16:p0 + 32, :], src[p0:p0 + 16, :])
        rtmp = sbB.tile([128, NP], BF16, name="rtmp", tag="rtmp", bufs=1)
        nc.vector.tensor_mul(dst[:], src[:], ctt[:])
        nc.vector.tensor_mul(rtmp[:], qp[:], sstt[:])
        nc.vector.tensor_add(dst[:], dst[:], rtmp[:])

    def qkv_closures(j):
        return [
            lambda: emit_qk_half(j, 0, 0),
            lambda: emit_qk_half(j, 0, 1),
            lambda: emit_qk_half(j, DIM, 0),
            lambda: emit_qk_half(j, DIM, 1),
            lambda: emit_rope(j, 0),
            lambda: emit_rope(j, 1),
        ]

    def norm_half(jj, half):
        c0 = half * 512
        rb = psB.tile([128, 512], F32, name="rb", tag="aux", bufs=2)
        nc.tensor.matmul(rb[:, :], selt[0:12, jj * 128:(jj + 1) * 128],
                         denrcb[0:12, c0:c0 + 512], start=True, stop=True)
        nc.vector.tensor_mul(oT[jj][:, c0:c0 + 512],
                             oT[jj][:, c0:c0 + 512], rb[:, :])

    def norm_closures(jj):
        # reciprocal of pair jj's denominators + rescale oT[jj]
        dsq = sbB.tile([128, 16], F32, name="dsq", tag="dsq", bufs=2)
        dsqr = sbB.tile([128, 16], BF16, name="dsqr", tag="dsqr", bufs=2)

        def c_fold():
            # quarter order must match denrcb row-major: hA|q0..1023, hB|...
            for (c0, h) in [(0, 0), (512, 0), (0, 1), (512, 1)]:
                q = h * 2 + (c0 // 512)
                nc.gpsimd.dma_start(dsq[q * 32:(q + 1) * 32, :],
                                    dnps[jj][(c0, h)][0:1, :])

        def c_recip():
            with nc.allow_low_precision(reason="bf16 softmax denominators"):
                nc.vector.reciprocal(dsqr[:, :], dsq[:, :])

        def c_unfold():
            nc.gpsimd.dma_start(denrcb[2 * jj:2 * jj + 2, 0:1024],
                                dsqr[:, :])

        return [c_fold, c_recip, c_unfold,
                lambda: norm_half(jj, 0),
                lambda: norm_half(jj, 1)]

    def emit_exp(it, eps, et):
        if it in EXP_DVE:
            nc.vector.tensor_scalar(et[:].bitcast(I16), eps[:],
                                    EXPA, EXPB, MULT, ADD)
        else:
            nc.scalar.activation(et[:], eps[:], EXP, scale=SCALE)

    # ---------- prologue: qkv(0), fixE-all, v projection ----------
    qkc0 = qkv_closures(0)
    for f in qkc0[:4]:
        f()
    # fixE for all heads: exp(q . k_cls) = exp((U^T x) * scale)
    fpsA = psB.tile([128, 1024], F32, name="fpsA", tag="eps", bufs=2)
    for c0 in (0, 512):
        for kc in range(6):
            nc.tensor.matmul(
                fpsA[0:12, c0:c0 + 512],
                ufixt[:, kc * 12:(kc + 1) * 12],
                xTt[kc][:, c0:c0 + 512],
                start=(kc == 0), stop=(kc == 5))
    nc.scalar.activation(et0A[:, :], fpsA[0:12, :], EXP, scale=SCALE)
    for h in range(H):
        nc.gpsimd.dma_start(et0w[0:1, h * NP:(h + 1) * NP], et0A[h:h + 1, :])
    for t in range(8):
        ps3 = psB.tile([128, 12, 64], F32, name="vps", tag="eps", bufs=2)
        for (h0, h1) in [(0, 8), (8, 12)]:
            for kc in range(6):
                nc.tensor.matmul(
                    ps3[:, h0:h1, :],
                    xTt[kc][:, t * 128:(t + 1) * 128],
                    wqt[kc][:, 2 * DIM + h0 * 64:2 * DIM + h1 * 64],
                    start=(kc == 0), stop=(kc == 5))
        nc.vector.tensor_copy(vA[t][:, :, 0:64], ps3[:, :, :])
        if t < 2:
            qkc0[4 + t]()

    # ---------- pair loop ----------
    dnps = [dict() for _ in range(6)]
    for j in range(6):
        hA, hB = 2 * j, 2 * j + 1
        bg = []
        if j >= 1:
            bg += norm_closures(j - 1)
        if j < 5:
            bg += qkv_closures(j + 1)

        ets = {}
        oaccs = {}
        gi = 0

        def extract_phase(c0, h, oacc_p, j=j):
            habs = 2 * j + h
            # cls-key contribution + stop
            nc.tensor.matmul(
                oacc_p[:, :],
                vclst[0:1, habs * 128:(habs + 1) * 128],
                et0w[0:1, habs * NP + c0:habs * NP + c0 + 512],
                start=False, stop=True, skip_group_check=True)
            row = 0 if h == 0 else 64
            nc.vector.tensor_copy(oT[j][row:row + 64, c0:c0 + 512],
                                  oacc_p[0:64, :])
            dnp = sbB.tile([1, 512], F32, name="dnp", tag="dnp", bufs=6)
            nc.vector.tensor_copy(dnp[0:1, :], oacc_p[64:65, 0:512])
            dnps[j][(c0, h)] = dnp

        def drain_job(j=j):
            nonlocal gi
            if gi >= 32:
                return
            g = gi
            gi += 1
            c0 = 0 if g < 16 else 512
            k = g % 16
            t, h = k // 2, k % 2
            habs = 2 * j + h
            if t == 0:
                oaccs[(c0, h)] = psB.tile([128, 512], F32, name="oacc",
                                          tag="oacc", bufs=2)
            oacc_p = oaccs[(c0, h)]
            et = ets[(t, c0)]
            nc.tensor.matmul(
                oacc_p[:, :], vA[t][:, habs, :], et[:, h * 512:h * 512 + 512],
                start=(t == 0), stop=False, skip_group_check=True)
            if t == 7:
                extract_phase(c0, h, oacc_p)

        it = 0
        for c0 in (0, 512):
            for t in range(8):
                eps = psB.tile([128, 1024], F32, name="eps", tag="eps",
                               bufs=2)
                nc.tensor.matmul(
                    eps[:, 0:512],
                    kTr[j][0:64, t * 128:(t + 1) * 128],
                    qTr[j][0:64, c0:c0 + 512],
                    start=True, stop=True)
                # full-row attnv between the two K=64 score matmuls keeps
                # them from row-group pairing, so the PE stays busier
                # (continuous-busy => max p-state) instead of idling.
                if it >= 2:
                    drain_job()
                nc.tensor.matmul(
                    eps[:, 512:1024],
                    kTr[j][64:128, t * 128:(t + 1) * 128],
                    qTr[j][64:128, c0:c0 + 512],
                    start=True, stop=True)
                if it >= 2:
                    drain_job()
                et = sbB.tile([128, 1024], BF16, name="et", tag="et", bufs=6)
                emit_exp(it, eps, et)
                ets[(t, c0)] = et
                if bg and 2 <= it <= 13:
                    bg.pop(0)()
                it += 1
        while bg:
            bg.pop(0)()
        while gi < 32:
            drain_job()

    # normalize last pair inline
    for f in norm_closures(5):
        f()

    psB_cm.__exit__(None, None, None)
    sbB_cm.__exit__(None, None, None)

    # ---------- tail: cls-query path + output projection ----------
    with tc.tile_pool(name="psD", bufs=1, space="PSUM") as psD, \
         tc.tile_pool(name="sbD", bufs=1) as sbD:
        clsps = psD.tile([128, 12], F32, name="clsps")
        clsrcp = main.tile([64, 12], F32, name="clsrcp")

        def emit_proj(tt):
            qoff, qw = (tt * 128, 128) if tt < 8 else (1024, 1)
            row0 = 1 + tt * 128 if tt < 8 else 0
            pr = psD.tile([128, DIM], F32, name="pr", tag="pr", bufs=2)
            for (c0, cw) in [(0, 512), (512, 256)]:
                for kc in range(6):
                    nc.tensor.matmul(
                        pr[:qw, c0:c0 + cw],
                        oT[kc][:, qoff:qoff + qw],
                        wpt[kc][:, c0:c0 + cw],
                        start=(kc == 0), stop=(kc == 5),
                        skip_group_check=True)
            osb = sbD.tile([128, DIM], F32, name="osb", tag="osb", bufs=2)
            nc.vector.tensor_add(osb[:qw, :], pr[:qw, :], biasf[:qw, :])
            nc.sync.dma_start(out[row0:row0 + qw, :], osb[:qw, :])

        def emit_clsscore(t):
            # psT[key, h] = sum_f x[f, key] * wfix[f, h]  (= q_cls . k_key)
            psT = psD.tile([128, 12], F32, name="psT", tag="psT", bufs=2)
            for kc in range(6):
                nc.tensor.matmul(
                    psT[:, 0:12],
                    xTt[kc][:, t * 128:(t + 1) * 128],
                    wfixt[:, kc * 12:(kc + 1) * 12],
                    start=(kc == 0), stop=(kc == 5))
            nc.scalar.activation(pTt[t][:], psT[:, 0:12], EXP, scale=SCALE)

        def emit_clsacc(h):
            for t in range(8):
                nc.tensor.matmul(clsps[:, h:h + 1],
                                 vA[t][:, h, :], pTt[t][:, h:h + 1],
                                 start=(t == 0), stop=False,
                                 skip_group_check=True)
            nc.tensor.matmul(clsps[:, h:h + 1],
                             vclst[0:1, h * 128:(h + 1) * 128],
                             pclst[0:1, h:h + 1],
                             start=False, stop=True, skip_group_check=True)

        emit_proj(0)
        emit_proj(1)
        for t in range(4):
            emit_clsscore(t)
        emit_proj(2)
        for t in range(4, 8):
            emit_clsscore(t)
        emit_proj(3)
        for h in range(6):
            emit_clsacc(h)
        emit_proj(4)
        for h in range(6, 12):
            emit_clsacc(h)
        emit_proj(5)
        emit_proj(6)
        emit_proj(7)
        nc.vector.reciprocal(clsrcp[:], clsps[64:128, :])
        for h in range(H):
            hj, hp = h // 2, 64 * (h % 2)
            nc.vector.tensor_mul(oT[hj][hp:hp + 64, 1024:1025],
                                 clsps[0:64, h:h + 1], clsrcp[:, h:h + 1])
        emit_proj(8)

    ctx.close()


def _build():
    nc = bacc.Bacc(trn_type="TRN2", target_bir_lowering=False)
    with tile.TileContext(nc) as tc:
        _build_body(tc)
    nc.finalize()
    return nc


def _host_tables(xpos_b):
    # cos/sin tables for patch tokens (1..1024), rows = 128 head-pair dims.
    py = xpos_b[1:, 0].astype(np.float64)
    px = xpos_b[1:, 1].astype(np.float64)
    inv = 1.0 / (100.0 ** (np.arange(0, 32, dtype=np.float64)[::2] / 32.0))
    angy = inv[:, None] * py[None, :]
    angx = inv[:, None] * px[None, :]
    c64 = np.concatenate([np.cos(angy), np.cos(angy), np.cos(angx), np.cos(angx)], 0)
    s64 = np.concatenate([np.sin(angy), np.sin(angy), np.sin(angx), np.sin(angx)], 0)
    c128 = np.concatenate([c64, c64], 0)
    s128 = np.concatenate([s64, s64], 0)
    # fold rotate-half signs into the sin table: rows (r%32)<16 negated
    r = np.arange(128)
    s128[(r % 32) < 16] *= -1.0
    bf = ml_dtypes.bfloat16
    return (np.ascontiguousarray(c128.astype(bf)),
            np.ascontiguousarray(s128.astype(bf)))


def kernel(**inputs):
    bf = ml_dtypes.bfloat16
    x = np.asarray(inputs["x"], np.float32)            # [8,1025,768]
    xpos = np.asarray(inputs["xpos"])                  # [8,1025,2]
    w_qkv = np.asarray(inputs["w_qkv"], np.float32)
    w_proj = np.asarray(inputs["w_proj"], np.float32)
    b_proj = np.asarray(inputs["b_proj"], np.float32).reshape(1, DIM)
    num_cls = int(np.asarray(inputs["num_cls"]))
    assert num_cls == 1, f"kernel specialized for num_cls=1, got {num_cls}"

    if "nc" not in _CACHE:
        _CACHE["nc"] = _build()
    nc = _CACHE["nc"]

    wq_bf = np.ascontiguousarray(w_qkv.T.astype(bf))       # [768, 2304]
    wp_bf = np.ascontiguousarray(w_proj.T.astype(bf))      # [768, 768]
    bp = np.ascontiguousarray(b_proj.astype(bf))
    sel = np.zeros((12, DIM), np.float32)
    for h in range(12):
        sel[h, h * 64:(h + 1) * 64] = 1.0
    sel = np.ascontiguousarray(sel.astype(bf))

    in_maps = []
    for b in range(NC):
        c128, s128 = _host_tables(xpos[b])
        xT = np.ascontiguousarray(x[b, 1:, :].T.astype(bf))  # [768, 1024]
        qkv0 = w_qkv @ x[b, 0, :]                            # [2304] cls qkv
        q0, k0, v0 = qkv0[0:DIM], qkv0[DIM:2 * DIM], qkv0[2 * DIM:3 * DIM]
        vcls = np.zeros((1, 12, 128), np.float32)
        for h in range(H):
            vcls[0, h, 0:64] = v0[h * 64:(h + 1) * 64]
            vcls[0, h, 64:128] = 1.0
        # ufix[:, h] = Wq_h^T k0_h  (fixE = ufix^T x = q . k_cls)
        # wfix[:, h] = Wk_h^T q0_h  (psT = x^T wfix = q_cls . k)
        ufix = np.zeros((DIM, 12), np.float32)
        wfix = np.zeros((DIM, 12), np.float32)
        for h in range(H):
            sl = slice(h * 64, (h + 1) * 64)
            ufix[:, h] = w_qkv[sl, :].T @ k0[sl]
            wfix[:, h] = w_qkv[DIM + h * 64:DIM + (h + 1) * 64, :].T @ q0[sl]
        # pack [768,12] -> [128, 72]: cols kc*12:(kc+1)*12 = chunk kc
        ufix_p = np.zeros((128, 72), np.float32)
        wfix_p = np.zeros((128, 72), np.float32)
        for kc in range(6):
            ufix_p[:, kc * 12:(kc + 1) * 12] = ufix[kc * 128:(kc + 1) * 128]
            wfix_p[:, kc * 12:(kc + 1) * 12] = wfix[kc * 128:(kc + 1) * 128]
        pcls = np.zeros((1, 12), np.float32)
        for h in range(H):
            pcls[0, h] = np.exp(
                float(q0[h * 64:(h + 1) * 64] @ k0[h * 64:(h + 1) * 64])
                * SCALE)
        in_maps.append({
            "xT": xT,
            "wq": wq_bf, "wp": wp_bf,
            "ct": c128, "sst": s128,
            "vcls": np.ascontiguousarray(vcls.reshape(1, 12 * 128).astype(bf)),
            "ufix": np.ascontiguousarray(ufix_p.astype(bf)),
            "wfix": np.ascontiguousarray(wfix_p.astype(bf)),
            "pcls": np.ascontiguousarray(pcls.astype(bf)),
            "sel": sel,
            "bp": bp,
        })
    res = run_bass_kernel_spmd(nc, in_maps, core_ids=list(range(NC)),
                               trace=bool(int(__import__("os").environ.get("BASS_TRACE_KERNEL", "0"))))
    _CACHE["last_result"] = res
    return np.stack([r["out"] for r in res.results], 0)


# revision 25
# speedup vs baseline: 1.1671x; 1.1671x over previous
"""Trainium2 Bass kernel: ViT attention block with 2D RoPE (croco-style).

Full inputs -> full outputs. Sharding: data-parallel over batch, one batch
element per NeuronCore (B=8 across 8 cores), no collectives.

v5c: v4 loop shape + surgical fixes.
  - attn@v lags TWO iterations behind scores (v4 lagged one), so attn@v
    never waits on the just-issued exp; kills the ~600ns/iter PE stall
    that kept the PE out of its fast p-state.
  - fixE (q . k_cls column) folded into the prologue: u_h = Wq_h^T k_cls
    on host, fixE rows = exp(U^T x) in one matmul set + one activation.
  - cls-query scores via w_h = Wk_h^T q_cls on host: psT = x^T W at the
    tail (no PE transposes); q_cls.k_cls shipped as exp from host.
  - Normalization inlined into the next pair: den row extracted per pair,
    folded [1,2048]->[128,16] by DMA so DVE reciprocal costs ~100ns
    (reciprocal is ~6.5ns per free-axis element), sel-matmul broadcast +
    one mul as background work.
  - proj bias via broadcast tensor_add instead of rank-1 matmuls.
  - exps all on ACT; gpsimd only does memsets/broadcast/cold input DMAs
    (GPSIMD cannot touch PSUM, and loading it slows the chip).
"""

import numpy as np
import ml_dtypes

import concourse.bass as bass
import concourse.mybir as mybir
import concourse.tile as tile
from concourse import bacc
from concourse.bass_utils import run_bass_kernel_spmd

F32 = mybir.dt.float32
BF16 = mybir.dt.bfloat16
EXP = mybir.ActivationFunctionType.Exp

DIM = 768
H = 12
HD = 64
N = 1025
NP = 1024
NC = 8
SCALE = HD ** -0.5

_CACHE = {}


def _build_body(tc):
    nc = tc.nc
    import contextlib
    ctx = contextlib.ExitStack()

    # ---- DRAM inputs (all host-prepped) ----
    xTd = nc.dram_tensor("xT", [DIM, NP], BF16, kind="ExternalInput")
    wqd = nc.dram_tensor("wq", [DIM, 3 * DIM], BF16, kind="ExternalInput")
    wpd = nc.dram_tensor("wp", [DIM, DIM], BF16, kind="ExternalInput")
    ctd = nc.dram_tensor("ct", [128, NP], BF16, kind="ExternalInput")
    sstd = nc.dram_tensor("sst", [128, NP], BF16, kind="ExternalInput")
    vclsd = nc.dram_tensor("vcls", [1, 12 * 128], BF16, kind="ExternalInput")
    ufixd = nc.dram_tensor("ufix", [128, 72], BF16, kind="ExternalInput")
    wfixd = nc.dram_tensor("wfix", [128, 72], BF16, kind="ExternalInput")
    pclsd = nc.dram_tensor("pcls", [1, 12], BF16, kind="ExternalInput")
    seld = nc.dram_tensor("sel", [12, DIM], BF16, kind="ExternalInput")
    bpd = nc.dram_tensor("bp", [1, DIM], BF16, kind="ExternalInput")
    out = nc.dram_tensor("out", [N, DIM], F32, kind="ExternalOutput")

    main = ctx.enter_context(tc.tile_pool(name="main", bufs=1))

    xTt = [main.tile([128, NP], BF16, name=f"xT{j}") for j in range(6)]
    wqt = [main.tile([128, 3 * DIM], BF16, name=f"wq{j}") for j in range(6)]
    wpt = [main.tile([128, DIM], BF16, name=f"wp{j}") for j in range(6)]
    ctt = main.tile([128, NP], BF16, name="ctt")
    sstt = main.tile([128, NP], BF16, name="sstt")
    vclst = main.tile([1, 12 * 128], BF16, name="vclst")
    ufixt = main.tile([128, 72], BF16, name="ufixt")
    wfixt = main.tile([128, 72], BF16, name="wfixt")
    pclst = main.tile([1, 12], BF16, name="pclst")
    selt = main.tile([12, DIM], BF16, name="selt")
    bpt = main.tile([1, DIM], BF16, name="bpt")
    biasf = main.tile([128, DIM], BF16, name="biasf")

    qT = [main.tile([128, NP], BF16, name=f"qT{j}") for j in range(6)]
    kT = [main.tile([128, NP], BF16, name=f"kT{j}") for j in range(6)]
    qTr = [main.tile([128, NP], BF16, name=f"qTr{j}") for j in range(6)]
    kTr = [main.tile([128, NP], BF16, name=f"kTr{j}") for j in range(6)]
    vA = [main.tile([128, 12, 128], BF16, name=f"vA{t}") for t in range(8)]
    oT = [main.tile([128, N], BF16, name=f"oT{j}") for j in range(6)]
    denrcb = main.tile([12, NP], BF16, name="denrcb")
    et0A = main.tile([12, NP], BF16, name="et0A")
    et0w = main.tile([1, 12 * NP], BF16, name="et0w")
    pTt = [main.tile([128, 12], BF16, name=f"pT{t}") for t in range(8)]

    # ---- input DMAs: hot set on sync queue (priority order) ----
    for kc in range(6):
        nc.sync.dma_start(xTt[kc][:], xTd[kc * 128:(kc + 1) * 128, :])
        nc.sync.dma_start(wqt[kc][:, 0:2 * DIM],
                          wqd[kc * 128:(kc + 1) * 128, 0:2 * DIM])
    for kc in range(6):
        nc.sync.dma_start(wqt[kc][:, 2 * DIM:3 * DIM],
                          wqd[kc * 128:(kc + 1) * 128, 2 * DIM:3 * DIM])
    nc.sync.dma_start(ctt[:], ctd[:])
    nc.sync.dma_start(sstt[:], sstd[:])
    # cold set on gpsimd queue (prologue-only use of gpsimd)
    nc.gpsimd.dma_start(ufixt[:], ufixd[:])
    nc.gpsimd.dma_start(vclst[:], vclsd[:])
    nc.gpsimd.dma_start(bpt[:], bpd[:])
    nc.gpsimd.dma_start(selt[:], seld[:])
    nc.gpsimd.dma_start(wfixt[:], wfixd[:])
    nc.gpsimd.dma_start(pclst[:], pclsd[:])
    for kc in range(6):
        nc.gpsimd.dma_start(wpt[kc][:], wpd[kc * 128:(kc + 1) * 128, :])

    for t in range(8):
        nc.gpsimd.memset(vA[t][:, :, 64:128], 1.0)
    nc.gpsimd.partition_broadcast(biasf[:], bpt[0:1, :])

    psB_cm = tc.tile_pool(name="psB", bufs=1, space="PSUM")
    psB = psB_cm.__enter__()
    sbB_cm = tc.tile_pool(name="sbB", bufs=1)
    sbB = sbB_cm.__enter__()

    # ---------- emission helpers ----------
    def emit_qk_half(j, sec, half, cell):
        # half 0: alloc psum + cols 0:512; half 1: cols 512:1024 + copy out
        if half == 0:
            cell[sec] = psB.tile([128, 1024], F32, name="qkps", tag="eps",
                                 bufs=2)
        ps = cell[sec]
        c0 = half * 512
        for kc in range(6):
            nc.tensor.matmul(
                ps[:, c0:c0 + 512],
                wqt[kc][:, sec + j * 128:sec + (j + 1) * 128],
                xTt[kc][:, c0:c0 + 512],
                start=(kc == 0), stop=(kc == 5))
        if half == 1:
            dst = qT[j] if sec == 0 else kT[j]
            nc.vector.tensor_copy(dst[:], ps[:])

    def emit_rope(j, which):
        src, dst = (qT[j], qTr[j]) if which == 0 else (kT[j], kTr[j])
        qp = sbB.tile([128, NP], BF16, name="qp", tag="qp", bufs=2)
        for b32 in range(4):
            p0 = b32 * 32
            nc.sync.dma_start(qp[p0:p0 + 16, :], src[p0 + 16:p0 + 32, :])
            nc.sync.dma_start(qp[p0 + 16:p0 + 32, :], src[p0:p0 + 16, :])
        rtmp = sbB.tile([128, NP], BF16, name="rtmp", tag="rtmp", bufs=2)
        nc.vector.tensor_mul(dst[:], src[:], ctt[:])
        nc.vector.tensor_mul(rtmp[:], qp[:], sstt[:])
        nc.vector.tensor_add(dst[:], dst[:], rtmp[:])

    def qkv_closures(j, cell):
        return [
            lambda: emit_qk_half(j, 0, 0, cell),
            lambda: emit_qk_half(j, 0, 1, cell),
            lambda: emit_qk_half(j, DIM, 0, cell),
            lambda: emit_qk_half(j, DIM, 1, cell),
            lambda: emit_rope(j, 0),
            lambda: emit_rope(j, 1),
        ]

    dns = [None] * 6

    def norm_closures(jj, pool=None, sb=None):
        # reciprocal of pair jj's denominators + rescale oT[jj]
        sbp = sb if sb is not None else sbB
        dsq = sbp.tile([128, 16], F32, name="dsq", tag="dsq", bufs=2)
        dsqr = sbp.tile([128, 16], BF16, name="dsqr", tag="dsqr", bufs=2)

        def c_fold():
            dnA, dnB = dns[jj]
            nc.sync.dma_start(dsq[0:64, :], dnA[0:1, :])
            nc.sync.dma_start(dsq[64:128, :], dnB[0:1, :])

        def c_recip():
            with nc.allow_low_precision(reason="bf16 softmax denominators"):
                nc.vector.reciprocal(dsqr[:, :], dsq[:, :])
            nc.sync.dma_start(denrcb[2 * jj:2 * jj + 2, 0:1024], dsqr[:, :])

        def c_rb():
            if pool is None:
                rb = psB.tile([128, 1024], F32, name="rb", tag="eps", bufs=2)
            else:
                rb = pool.tile([128, 1024], F32, name="rbT", tag="rbT",
                               bufs=1)
            for c0 in (0, 512):
                nc.tensor.matmul(rb[:, c0:c0 + 512],
                                 selt[0:12, jj * 128:(jj + 1) * 128],
                                 denrcb[0:12, c0:c0 + 512],
                                 start=True, stop=True)
            nc.vector.tensor_mul(oT[jj][:, 0:1024], oT[jj][:, 0:1024],
                                 rb[:, :])

        return [c_fold, c_recip, c_rb]

    def emit_vproj(t):
        ps3 = psB.tile([128, 12, 64], F32, name="vps", tag="eps", bufs=2)
        for (h0, h1) in [(0, 8), (8, 12)]:
            for kc in range(6):
                nc.tensor.matmul(
                    ps3[:, h0:h1, :],
                    xTt[kc][:, t * 128:(t + 1) * 128],
                    wqt[kc][:, 2 * DIM + h0 * 64:2 * DIM + h1 * 64],
                    start=(kc == 0), stop=(kc == 5))
        nc.vector.tensor_copy(vA[t][:, :, 0:64], ps3[:, :, :])

    def emit_fixA():
        # fixE for all heads: exp((U^T x) * scale) = exp(q . k_cls * scale)
        fpsA = psB.tile([128, 1024], F32, name="fpsA", tag="eps", bufs=2)
        for c0 in (0, 512):
            for kc in range(6):
                nc.tensor.matmul(
                    fpsA[0:12, c0:c0 + 512],
                    ufixt[:, kc * 12:(kc + 1) * 12],
                    xTt[kc][:, c0:c0 + 512],
                    start=(kc == 0), stop=(kc == 5))
        nc.scalar.activation(et0A[:, :], fpsA[0:12, :], EXP, scale=SCALE)
        for h in range(H):
            nc.sync.dma_start(et0w[0:1, h * NP:(h + 1) * NP],
                              et0A[h:h + 1, :])

    # ---------- prologue: just qkv(0) + rope(0) ----------
    cells = [dict() for _ in range(6)]
    qkc0 = qkv_closures(0, cells[0])
    for f in qkc0:
        f()

    # ---------- pair loop, software-pipelined ----------
    for j in range(6):
        hA, hB = 2 * j, 2 * j + 1
        bg = []
        if j == 0:
            # fat pair 0: v-projection, fixE-all and pair-1 qkv as bg
            bg += [lambda t=t: emit_vproj(t) for t in range(8)]
            bg += [emit_fixA]
            bg += qkv_closures(1, cells[1])
            bg_lo, bg_hi = 0, 14
        else:
            bg += norm_closures(j - 1)
            if j < 5:
                bg += qkv_closures(j + 1, cells[j + 1])
            bg_lo, bg_hi = 2, 10
        oaccA = psB.tile([128, 1024], F32, name="oaccA", tag="oacc", bufs=2)
        oaccB = psB.tile([128, 1024], F32, name="oaccB", tag="oacc", bufs=2)
        hist = []
        it = 0

        def attnv(pet, pt, pc0, oA=oaccA, oB=oaccB, hAx=hA, hBx=hB):
            nc.tensor.matmul(
                oA[:, pc0:pc0 + 512],
                vA[pt][:, hAx, :], pet[:, 0:512],
                start=(pt == 0), stop=False, skip_group_check=True)
            nc.tensor.matmul(
                oB[:, pc0:pc0 + 512],
                vA[pt][:, hBx, :], pet[:, 512:1024],
                start=(pt == 0), stop=False, skip_group_check=True)

        for t in range(8):
            for c0 in (0, 512):
                eps = psB.tile([128, 1024], F32, name="eps", tag="eps",
                               bufs=2)
                nc.tensor.matmul(
                    eps[:, 0:512],
                    kTr[j][0:64, t * 128:(t + 1) * 128],
                    qTr[j][0:64, c0:c0 + 512],
                    start=True, stop=True)
                nc.tensor.matmul(
                    eps[:, 512:1024],
                    kTr[j][64:128, t * 128:(t + 1) * 128],
                    qTr[j][64:128, c0:c0 + 512],
                    start=True, stop=True)
                et = sbB.tile([128, 1024], BF16, name="et", tag="et", bufs=5)
                nc.scalar.activation(et[:, :], eps[:, :], EXP, scale=SCALE)
                if bg and bg_lo <= it <= bg_hi:
                    bg.pop(0)()
                hist.append((et, t, c0))
                if len(hist) > 2:
                    attnv(*hist.pop(0))
                it += 1
        while bg:
            bg.pop(0)()
        for args in hist:
            attnv(*args)
        hist.clear()
        # head A: cls-key stops, then extract (releases oaccA first)
        for c0 in (0, 512):
            nc.tensor.matmul(
                oaccA[:, c0:c0 + 512],
                vclst[0:1, hA * 128:(hA + 1) * 128],
                et0w[0:1, hA * NP + c0:hA * NP + c0 + 512],
                start=False, stop=True, skip_group_check=True)
        dnA = sbB.tile([1, 1024], F32, name="dnA", tag="dn", bufs=4)
        nc.vector.tensor_copy(oT[j][0:64, 0:1024], oaccA[0:64, :])
        nc.vector.tensor_copy(dnA[:], oaccA[64:65, :])
        for c0 in (0, 512):
            nc.tensor.matmul(
                oaccB[:, c0:c0 + 512],
                vclst[0:1, hB * 128:(hB + 1) * 128],
                et0w[0:1, hB * NP + c0:hB * NP + c0 + 512],
                start=False, stop=True, skip_group_check=True)
        dnB = sbB.tile([1, 1024], F32, name="dnB", tag="dn", bufs=4)
        nc.vector.tensor_copy(oT[j][64:128, 0:1024], oaccB[0:64, :])
        nc.vector.tensor_copy(dnB[:], oaccB[64:65, :])
        dns[j] = (dnA, dnB)

    psB_cm.__exit__(None, None, None)
    sbB_cm.__exit__(None, None, None)

    # ---------- tail: cls-query path + output projection ----------
    with tc.tile_pool(name="psD", bufs=1, space="PSUM") as psD, \
         tc.tile_pool(name="sbD", bufs=1) as sbD:
        clsps = psD.tile([128, 12], F32, name="clsps")
        clsrcp = main.tile([64, 12], F32, name="clsrcp")

        def emit_proj(tt, kcs=range(6), pr=None, finish=True):
            qoff, qw = (tt * 128, 128) if tt < 8 else (1024, 1)
            row0 = 1 + tt * 128 if tt < 8 else 0
            if pr is None:
                pr = psD.tile([128, DIM], F32, name="pr", tag="pr", bufs=2)
            for (c0, cw) in [(0, 512), (512, 256)]:
                for kc in kcs:
                    nc.tensor.matmul(
                        pr[:qw, c0:c0 + cw],
                        oT[kc][:, qoff:qoff + qw],
                        wpt[kc][:, c0:c0 + cw],
                        start=(kc == 0), stop=(kc == 5),
                        skip_group_check=True)
            if not finish:
                return pr
            osb = sbD.tile([128, DIM], F32, name="osb", tag="osb", bufs=2)
            nc.vector.tensor_add(osb[:qw, :], pr[:qw, :], biasf[:qw, :])
            nc.sync.dma_start(out[row0:row0 + qw, :], osb[:qw, :])
            return pr

        def emit_clsscore(t):
            # psT[key, h] = sum_f x[f, key] * wfix[f, h]  (= q_cls . k_key)
            psT = psD.tile([128, 12], F32, name="psT", tag="psT", bufs=1)
            for kc in range(6):
                nc.tensor.matmul(
                    psT[:, 0:12],
                    xTt[kc][:, t * 128:(t + 1) * 128],
                    wfixt[:, kc * 12:(kc + 1) * 12],
                    start=(kc == 0), stop=(kc == 5))
            nc.scalar.activation(pTt[t][:], psT[:, 0:12], EXP, scale=SCALE)

        def emit_clsacc(h):
            for t in range(8):
                nc.tensor.matmul(clsps[:, h:h + 1],
                                 vA[t][:, h, :], pTt[t][:, h:h + 1],
                                 start=(t == 0), stop=False,
                                 skip_group_check=True)
            nc.tensor.matmul(clsps[:, h:h + 1],
                             vclst[0:1, h * 128:(h + 1) * 128],
                             pclst[0:1, h:h + 1],
                             start=False, stop=True, skip_group_check=True)

        # proj tiles 0/1 contract kc 0..4 while pair-5 normalization (which
        # gates oT[5]) completes in the background; kc=5 is deferred.
        pr0 = emit_proj(0, kcs=range(5), finish=False)
        pr1 = emit_proj(1, kcs=range(5), finish=False)
        for f in norm_closures(5, pool=psD, sb=sbD):
            f()
        emit_proj(0, kcs=[5], pr=pr0)
        emit_proj(1, kcs=[5], pr=pr1)
        for t in range(4):
            emit_clsscore(t)
        emit_proj(2)
        for t in range(4, 8):
            emit_clsscore(t)
        emit_proj(3)
        for h in range(6):
            emit_clsacc(h)
        emit_proj(4)
        for h in range(6, 12):
            emit_clsacc(h)
        emit_proj(5)
        emit_proj(6)
        emit_proj(7)
        nc.vector.reciprocal(clsrcp[:], clsps[64:128, :])
        for h in range(H):
            hj, hp = h // 2, 64 * (h % 2)
            nc.vector.tensor_mul(oT[hj][hp:hp + 64, 1024:1025],
                                 clsps[0:64, h:h + 1], clsrcp[:, h:h + 1])
        emit_proj(8)

    ctx.close()


def _build():
    nc = bacc.Bacc(trn_type="TRN2", target_bir_lowering=False)
    with tile.TileContext(nc) as tc:
        _build_body(tc)
    nc.finalize()
    return nc


def _host_tables(xpos_b):
    # cos/sin tables for patch tokens (1..1024), rows = 128 head-pair dims.
    py = xpos_b[1:, 0].astype(np.float64)
    px = xpos_b[1:, 1].astype(np.float64)
    inv = 1.0 / (100.0 ** (np.arange(0, 32, dtype=np.float64)[::2] / 32.0))
    angy = inv[:, None] * py[None, :]
    angx = inv[:, None] * px[None, :]
    c64 = np.concatenate([np.cos(angy), np.cos(angy), np.cos(angx), np.cos(angx)], 0)
    s64 = np.concatenate([np.sin(angy), np.sin(angy), np.sin(angx), np.sin(angx)], 0)
    c128 = np.concatenate([c64, c64], 0)
    s128 = np.concatenate([s64, s64], 0)
    # fold rotate-half signs into the sin table: rows (r%32)<16 negated
    r = np.arange(128)
    s128[(r % 32) < 16] *= -1.0
    bf = ml_dtypes.bfloat16
    return (np.ascontiguousarray(c128.astype(bf)),
            np.ascontiguousarray(s128.astype(bf)))


def kernel(**inputs):
    bf = ml_dtypes.bfloat16
    x = np.asarray(inputs["x"], np.float32)            # [8,1025,768]
    xpos = np.asarray(inputs["xpos"])                  # [8,1025,2]
    w_qkv = np.asarray(inputs["w_qkv"], np.float32)
    w_proj = np.asarray(inputs["w_proj"], np.float32)
    b_proj = np.asarray(inputs["b_proj"], np.float32).reshape(1, DIM)
    num_cls = int(np.asarray(inputs["num_cls"]))
    assert num_cls == 1, f"kernel specialized for num_cls=1, got {num_cls}"

    if "nc" not in _CACHE:
        _CACHE["nc"] = _build()
    nc = _CACHE["nc"]

    wq_bf = np.ascontiguousarray(w_qkv.T.astype(bf))       # [768, 2304]
    wp_bf = np.ascontiguousarray(w_proj.T.astype(bf))      # [768, 768]
    bp = np.ascontiguousarray(b_proj.astype(bf))
    sel = np.zeros((12, DIM), np.float32)
    for h in range(12):
        sel[h, h * 64:(h + 1) * 64] = 1.0
    sel = np.ascontiguousarray(sel.astype(bf))

    in_maps = []
    for b in range(NC):
        c128, s128 = _host_tables(xpos[b])
        xT = np.ascontiguousarray(x[b, 1:, :].T.astype(bf))  # [768, 1024]
        qkv0 = w_qkv @ x[b, 0, :]                            # [2304] cls qkv
        q0, k0, v0 = qkv0[0:DIM], qkv0[DIM:2 * DIM], qkv0[2 * DIM:3 * DIM]
        vcls = np.zeros((1, 12, 128), np.float32)
        for h in range(H):
            vcls[0, h, 0:64] = v0[h * 64:(h + 1) * 64]
            vcls[0, h, 64:128] = 1.0
        # ufix[:, h] = Wq_h^T k0_h  (fixE = ufix^T x = q . k_cls)
        # wfix[:, h] = Wk_h^T q0_h  (psT = x^T wfix = q_cls . k)
        ufix = np.zeros((DIM, 12), np.float32)
        wfix = np.zeros((DIM, 12), np.float32)
        for h in range(H):
            sl = slice(h * 64, (h + 1) * 64)
            ufix[:, h] = w_qkv[sl, :].T @ k0[sl]
            wfix[:, h] = w_qkv[DIM + h * 64:DIM + (h + 1) * 64, :].T @ q0[sl]
        ufix_p = np.zeros((128, 72), np.float32)
        wfix_p = np.zeros((128, 72), np.float32)
        for kc in range(6):
            ufix_p[:, kc * 12:(kc + 1) * 12] = ufix[kc * 128:(kc + 1) * 128]
            wfix_p[:, kc * 12:(kc + 1) * 12] = wfix[kc * 128:(kc + 1) * 128]
        pcls = np.zeros((1, 12), np.float32)
        for h in range(H):
            pcls[0, h] = np.exp(
                float(q0[h * 64:(h + 1) * 64] @ k0[h * 64:(h + 1) * 64])
                * SCALE)
        in_maps.append({
            "xT": xT,
            "wq": wq_bf, "wp": wp_bf,
            "ct": c128, "sst": s128,
            "vcls": np.ascontiguousarray(vcls.reshape(1, 12 * 128).astype(bf)),
            "ufix": np.ascontiguousarray(ufix_p.astype(bf)),
            "wfix": np.ascontiguousarray(wfix_p.astype(bf)),
            "pcls": np.ascontiguousarray(pcls.astype(bf)),
            "sel": sel,
            "bp": bp,
        })
    res = run_bass_kernel_spmd(nc, in_maps, core_ids=list(range(NC)),
                               trace=bool(int(__import__("os").environ.get("BASS_TRACE_KERNEL", "0"))))
    _CACHE["last_result"] = res
    return np.stack([r["out"] for r in res.results], 0)


# revision 27
# speedup vs baseline: 1.3507x; 1.1573x over previous
"""Trainium2 Bass kernel: ViT attention block with 2D RoPE (croco-style).

Full inputs -> full outputs. Sharding: data-parallel over batch, one batch
element per NeuronCore (B=8 across 8 cores), no collectives.

v5c: v4 loop shape + surgical fixes.
  - attn@v lags TWO iterations behind scores (v4 lagged one), so attn@v
    never waits on the just-issued exp; kills the ~600ns/iter PE stall
    that kept the PE out of its fast p-state.
  - fixE (q . k_cls column) folded into the prologue: u_h = Wq_h^T k_cls
    on host, fixE rows = exp(U^T x) in one matmul set + one activation.
  - cls-query scores via w_h = Wk_h^T q_cls on host: psT = x^T W at the
    tail (no PE transposes); q_cls.k_cls shipped as exp from host.
  - Normalization inlined into the next pair: den row extracted per pair,
    folded [1,2048]->[128,16] by DMA so DVE reciprocal costs ~100ns
    (reciprocal is ~6.5ns per free-axis element), sel-matmul broadcast +
    one mul as background work.
  - proj bias via broadcast tensor_add instead of rank-1 matmuls.
  - exps all on ACT; gpsimd only does memsets/broadcast/cold input DMAs
    (GPSIMD cannot touch PSUM, and loading it slows the chip).
"""

import numpy as np
import ml_dtypes

import concourse.bass as bass
import concourse.mybir as mybir
import concourse.tile as tile
from concourse import bacc
from concourse.bass_utils import run_bass_kernel_spmd

F32 = mybir.dt.float32
BF16 = mybir.dt.bfloat16
EXP = mybir.ActivationFunctionType.Exp

DIM = 768
H = 12
HD = 64
N = 1025
NP = 1024
NC = 8
SCALE = HD ** -0.5

_CACHE = {}


def _build_body(tc):
    nc = tc.nc
    import contextlib
    ctx = contextlib.ExitStack()

    # ---- DRAM inputs (all host-prepped) ----
    xTd = nc.dram_tensor("xT", [DIM, NP], BF16, kind="ExternalInput")
    wqd = nc.dram_tensor("wq", [DIM, 3 * DIM], BF16, kind="ExternalInput")
    wpd = nc.dram_tensor("wp", [DIM, DIM], BF16, kind="ExternalInput")
    ctd = nc.dram_tensor("ct", [128, NP], BF16, kind="ExternalInput")
    sstd = nc.dram_tensor("sst", [128, NP], BF16, kind="ExternalInput")
    vclsd = nc.dram_tensor("vcls", [1, 12 * 128], BF16, kind="ExternalInput")
    ufixd = nc.dram_tensor("ufix", [128, 72], BF16, kind="ExternalInput")
    wfixd = nc.dram_tensor("wfix", [128, 72], BF16, kind="ExternalInput")
    pclsd = nc.dram_tensor("pcls", [1, 12], BF16, kind="ExternalInput")
    seld = nc.dram_tensor("sel", [12, DIM], BF16, kind="ExternalInput")
    bpd = nc.dram_tensor("bp", [1, DIM], BF16, kind="ExternalInput")
    out = nc.dram_tensor("out", [N, DIM], F32, kind="ExternalOutput")

    main = ctx.enter_context(tc.tile_pool(name="main", bufs=1))

    xTt = [main.tile([128, NP], BF16, name=f"xT{j}") for j in range(6)]
    wqt = [main.tile([128, 3 * DIM], BF16, name=f"wq{j}") for j in range(6)]
    wpt = [main.tile([128, DIM], BF16, name=f"wp{j}") for j in range(6)]
    ctt = main.tile([128, NP], BF16, name="ctt")
    sstt = main.tile([128, NP], BF16, name="sstt")
    vclst = main.tile([1, 12 * 128], BF16, name="vclst")
    ufixt = main.tile([128, 72], BF16, name="ufixt")
    wfixt = main.tile([128, 72], BF16, name="wfixt")
    pclst = main.tile([1, 12], BF16, name="pclst")
    selt = main.tile([12, DIM], BF16, name="selt")
    bpt = main.tile([1, DIM], BF16, name="bpt")
    biasf = main.tile([128, DIM], BF16, name="biasf")

    qT = [main.tile([128, NP], BF16, name=f"qT{j}") for j in range(6)]
    kT = [main.tile([128, NP], BF16, name=f"kT{j}") for j in range(6)]
    qTr = [main.tile([128, NP], BF16, name=f"qTr{j}") for j in range(6)]
    kTr = [main.tile([128, NP], BF16, name=f"kTr{j}") for j in range(6)]
    vA = [main.tile([128, 12, 128], BF16, name=f"vA{t}") for t in range(8)]
    oT = [main.tile([128, N], BF16, name=f"oT{j}") for j in range(6)]
    denrcb = main.tile([12, NP], BF16, name="denrcb")
    et0A = main.tile([12, NP], BF16, name="et0A")
    et0w = main.tile([1, 12 * NP], BF16, name="et0w")
    pTt = [main.tile([128, 12], BF16, name=f"pT{t}") for t in range(8)]

    # ---- input DMAs: hot set on sync queue (priority order) ----
    for kc in range(6):
        nc.sync.dma_start(xTt[kc][:], xTd[kc * 128:(kc + 1) * 128, :])
        nc.sync.dma_start(wqt[kc][:, 0:2 * DIM],
                          wqd[kc * 128:(kc + 1) * 128, 0:2 * DIM])
    for kc in range(6):
        nc.sync.dma_start(wqt[kc][:, 2 * DIM:3 * DIM],
                          wqd[kc * 128:(kc + 1) * 128, 2 * DIM:3 * DIM])
    nc.sync.dma_start(ctt[:], ctd[:])
    nc.sync.dma_start(sstt[:], sstd[:])
    # cold set on gpsimd queue (prologue-only use of gpsimd)
    nc.gpsimd.dma_start(ufixt[:], ufixd[:])
    nc.gpsimd.dma_start(vclst[:], vclsd[:])
    nc.gpsimd.dma_start(bpt[:], bpd[:])
    nc.gpsimd.dma_start(selt[:], seld[:])
    nc.gpsimd.dma_start(wfixt[:], wfixd[:])
    nc.gpsimd.dma_start(pclst[:], pclsd[:])
    for kc in range(6):
        nc.gpsimd.dma_start(wpt[kc][:], wpd[kc * 128:(kc + 1) * 128, :])

    for t in range(8):
        nc.gpsimd.memset(vA[t][:, :, 64:128], 1.0)
    nc.gpsimd.partition_broadcast(biasf[:], bpt[0:1, :])

    psB_cm = tc.tile_pool(name="psB", bufs=1, space="PSUM")
    psB = psB_cm.__enter__()
    sbB_cm = tc.tile_pool(name="sbB", bufs=1)
    sbB = sbB_cm.__enter__()

    # ---------- emission helpers ----------
    def emit_qk_half(j, sec, half, cell):
        # half 0: alloc psum + cols 0:512; half 1: cols 512:1024 + copy out
        if half == 0:
            cell[sec] = psB.tile([128, 1024], F32, name="qkps", tag="eps",
                                 bufs=2)
        ps = cell[sec]
        c0 = half * 512
        for kc in range(6):
            nc.tensor.matmul(
                ps[:, c0:c0 + 512],
                wqt[kc][:, sec + j * 128:sec + (j + 1) * 128],
                xTt[kc][:, c0:c0 + 512],
                start=(kc == 0), stop=(kc == 5))
        if half == 1:
            dst = qT[j] if sec == 0 else kT[j]
            nc.scalar.copy(dst[:], ps[:])

    def emit_rope(j, which):
        src, dst = (qT[j], qTr[j]) if which == 0 else (kT[j], kTr[j])
        qp = sbB.tile([128, NP], BF16, name="qp", tag="qp", bufs=2)
        for b32 in range(4):
            p0 = b32 * 32
            nc.gpsimd.dma_start(qp[p0:p0 + 16, :], src[p0 + 16:p0 + 32, :])
            nc.gpsimd.dma_start(qp[p0 + 16:p0 + 32, :], src[p0:p0 + 16, :])
        rtmp = sbB.tile([128, NP], BF16, name="rtmp", tag="rtmp", bufs=2)
        nc.vector.tensor_mul(dst[:], src[:], ctt[:])
        nc.vector.tensor_mul(rtmp[:], qp[:], sstt[:])
        nc.vector.tensor_add(dst[:], dst[:], rtmp[:])

    def qkv_closures(j, cell):
        return [
            lambda: emit_qk_half(j, 0, 0, cell),
            lambda: emit_qk_half(j, 0, 1, cell),
            lambda: emit_qk_half(j, DIM, 0, cell),
            lambda: emit_qk_half(j, DIM, 1, cell),
            lambda: emit_rope(j, 0),
            lambda: emit_rope(j, 1),
        ]

    dns = [None] * 6

    def norm_closures(jj, pool=None, sb=None):
        # reciprocal of pair jj's denominators + rescale oT[jj]
        sbp = sb if sb is not None else sbB
        dsq = sbp.tile([128, 16], F32, name="dsq", tag="dsq", bufs=2)
        dsqr = sbp.tile([128, 16], BF16, name="dsqr", tag="dsqr", bufs=2)

        def c_fold():
            dnA, dnB = dns[jj]
            nc.sync.dma_start(dsq[0:64, :], dnA[0:1, :])
            nc.sync.dma_start(dsq[64:128, :], dnB[0:1, :])

        def c_recip():
            with nc.allow_low_precision(reason="bf16 softmax denominators"):
                nc.vector.reciprocal(dsqr[:, :], dsq[:, :])
            nc.sync.dma_start(denrcb[2 * jj:2 * jj + 2, 0:1024], dsqr[:, :])

        def c_rb():
            if pool is None:
                rb = psB.tile([128, 1024], F32, name="rb", tag="eps", bufs=2)
            else:
                rb = pool.tile([128, 1024], F32, name="rbT", tag="rbT",
                               bufs=1)
            for c0 in (0, 512):
                nc.tensor.matmul(rb[:, c0:c0 + 512],
                                 selt[0:12, jj * 128:(jj + 1) * 128],
                                 denrcb[0:12, c0:c0 + 512],
                                 start=True, stop=True)
            nc.vector.tensor_mul(oT[jj][:, 0:1024], oT[jj][:, 0:1024],
                                 rb[:, :])

        return [c_fold, c_recip, c_rb]

    def emit_vproj(t):
        ps3 = psB.tile([128, 12, 64], F32, name="vps", tag="eps", bufs=2)
        for (h0, h1) in [(0, 8), (8, 12)]:
            for kc in range(6):
                nc.tensor.matmul(
                    ps3[:, h0:h1, :],
                    xTt[kc][:, t * 128:(t + 1) * 128],
                    wqt[kc][:, 2 * DIM + h0 * 64:2 * DIM + h1 * 64],
                    start=(kc == 0), stop=(kc == 5))
        nc.scalar.copy(vA[t][:, :, 0:64], ps3[:, :, :])

    def emit_fixA():
        # fixE for all heads: exp((U^T x) * scale) = exp(q . k_cls * scale)
        fpsA = psB.tile([128, 1024], F32, name="fpsA", tag="eps", bufs=2)
        for c0 in (0, 512):
            for kc in range(6):
                nc.tensor.matmul(
                    fpsA[0:12, c0:c0 + 512],
                    ufixt[:, kc * 12:(kc + 1) * 12],
                    xTt[kc][:, c0:c0 + 512],
                    start=(kc == 0), stop=(kc == 5))
        nc.scalar.activation(et0A[:, :], fpsA[0:12, :], EXP, scale=SCALE)
        for h in range(H):
            nc.gpsimd.dma_start(et0w[0:1, h * NP:(h + 1) * NP],
                                et0A[h:h + 1, :])

    # ---------- prologue: ALL qkv + v-proj + fixE (ACT-quiet, PE fast) ----
    cells = [dict() for _ in range(6)]
    qkcs = [qkv_closures(j, cells[j]) for j in range(6)]
    vps = [(lambda t=t: emit_vproj(t)) for t in range(8)]
    prolog = (qkcs[0][:4] + [vps[0], vps[1]] + qkcs[1][:4]
              + [vps[2], vps[3]] + qkcs[2][:4] + [vps[4], vps[5]]
              + qkcs[3][:4] + [vps[6], vps[7]] + qkcs[4][:4]
              + [emit_fixA] + qkcs[5][:4])
    for f in prolog:
        f()
    qkcs[0][4]()
    qkcs[0][5]()
    qkcs[1][4]()
    qkcs[1][5]()

    # ---------- pair loop, software-pipelined ----------
    for j in range(6):
        hA, hB = 2 * j, 2 * j + 1
        bg = []
        if 1 <= j <= 4:
            bg += [qkcs[j + 1][4], qkcs[j + 1][5]]  # rope(j+1)
        if j >= 1:
            bg += norm_closures(j - 1)
        bg_lo, bg_hi = 2, 10
        oaccA = psB.tile([128, 1024], F32, name="oaccA", tag="oacc", bufs=2)
        oaccB = psB.tile([128, 1024], F32, name="oaccB", tag="oacc", bufs=2)
        hist = []
        it = 0

        def attnv(pet, pt, pc0, oA=oaccA, oB=oaccB, hAx=hA, hBx=hB):
            nc.tensor.matmul(
                oA[:, pc0:pc0 + 512],
                vA[pt][:, hAx, :], pet[:, 0:512],
                start=(pt == 0), stop=False, skip_group_check=True)
            nc.tensor.matmul(
                oB[:, pc0:pc0 + 512],
                vA[pt][:, hBx, :], pet[:, 512:1024],
                start=(pt == 0), stop=False, skip_group_check=True)

        for t in range(8):
            for c0 in (0, 512):
                eps = psB.tile([128, 1024], F32, name="eps", tag="eps",
                               bufs=2)
                nc.tensor.matmul(
                    eps[:, 0:512],
                    kTr[j][0:64, t * 128:(t + 1) * 128],
                    qTr[j][0:64, c0:c0 + 512],
                    start=True, stop=True)
                nc.tensor.matmul(
                    eps[:, 512:1024],
                    kTr[j][64:128, t * 128:(t + 1) * 128],
                    qTr[j][64:128, c0:c0 + 512],
                    start=True, stop=True)
                et = sbB.tile([128, 1024], BF16, name="et", tag="et", bufs=5)
                nc.scalar.activation(et[:, :], eps[:, :], EXP, scale=SCALE)
                if bg and bg_lo <= it <= bg_hi:
                    bg.pop(0)()
                hist.append((et, t, c0))
                if len(hist) > 2:
                    attnv(*hist.pop(0))
                it += 1
        while bg:
            bg.pop(0)()
        for args in hist:
            attnv(*args)
        hist.clear()
        # head A: cls-key stops, then extract (releases oaccA first)
        for c0 in (0, 512):
            nc.tensor.matmul(
                oaccA[:, c0:c0 + 512],
                vclst[0:1, hA * 128:(hA + 1) * 128],
                et0w[0:1, hA * NP + c0:hA * NP + c0 + 512],
                start=False, stop=True, skip_group_check=True)
        dnA = sbB.tile([1, 1024], F32, name="dnA", tag="dn", bufs=4)
        nc.scalar.copy(oT[j][0:64, 0:1024], oaccA[0:64, :])
        nc.vector.tensor_copy(dnA[:], oaccA[64:65, :])
        for c0 in (0, 512):
            nc.tensor.matmul(
                oaccB[:, c0:c0 + 512],
                vclst[0:1, hB * 128:(hB + 1) * 128],
                et0w[0:1, hB * NP + c0:hB * NP + c0 + 512],
                start=False, stop=True, skip_group_check=True)
        dnB = sbB.tile([1, 1024], F32, name="dnB", tag="dn", bufs=4)
        nc.scalar.copy(oT[j][64:128, 0:1024], oaccB[0:64, :])
        nc.vector.tensor_copy(dnB[:], oaccB[64:65, :])
        dns[j] = (dnA, dnB)

    psB_cm.__exit__(None, None, None)
    sbB_cm.__exit__(None, None, None)

    # ---------- tail: cls-query path + output projection ----------
    with tc.tile_pool(name="psD", bufs=1, space="PSUM") as psD, \
         tc.tile_pool(name="sbD", bufs=1) as sbD:
        clsps = psD.tile([128, 12], F32, name="clsps")
        clsrcp = main.tile([64, 12], F32, name="clsrcp")

        def emit_proj(tt, kcs=range(6), pr=None, finish=True):
            qoff, qw = (tt * 128, 128) if tt < 8 else (1024, 1)
            row0 = 1 + tt * 128 if tt < 8 else 0
            if pr is None:
                pr = psD.tile([128, DIM], F32, name="pr", tag="pr", bufs=2)
            for (c0, cw) in [(0, 512), (512, 256)]:
                for kc in kcs:
                    nc.tensor.matmul(
                        pr[:qw, c0:c0 + cw],
                        oT[kc][:, qoff:qoff + qw],
                        wpt[kc][:, c0:c0 + cw],
                        start=(kc == 0), stop=(kc == 5),
                        skip_group_check=True)
            if not finish:
                return pr
            osb = sbD.tile([128, DIM], F32, name="osb", tag="osb", bufs=2)
            nc.vector.tensor_add(osb[:qw, :], pr[:qw, :], biasf[:qw, :])
            nc.sync.dma_start(out[row0:row0 + qw, :], osb[:qw, :])
            return pr

        def emit_clsscore(t):
            # psT[key, h] = sum_f x[f, key] * wfix[f, h]  (= q_cls . k_key)
            psT = psD.tile([128, 12], F32, name="psT", tag="psT", bufs=1)
            for kc in range(6):
                nc.tensor.matmul(
                    psT[:, 0:12],
                    xTt[kc][:, t * 128:(t + 1) * 128],
                    wfixt[:, kc * 12:(kc + 1) * 12],
                    start=(kc == 0), stop=(kc == 5))
            nc.scalar.activation(pTt[t][:], psT[:, 0:12], EXP, scale=SCALE)

        def emit_clsacc(h):
            for t in range(8):
                nc.tensor.matmul(clsps[:, h:h + 1],
                                 vA[t][:, h, :], pTt[t][:, h:h + 1],
                                 start=(t == 0), stop=False,
                                 skip_group_check=True)
            nc.tensor.matmul(clsps[:, h:h + 1],
                             vclst[0:1, h * 128:(h + 1) * 128],
                             pclst[0:1, h:h + 1],
                             start=False, stop=True, skip_group_check=True)

        # proj tiles 0/1 contract kc 0..4 while pair-5 normalization (which
        # gates oT[5]) completes in the background; kc=5 is deferred.
        pr0 = emit_proj(0, kcs=range(5), finish=False)
        pr1 = emit_proj(1, kcs=range(5), finish=False)
        for f in norm_closures(5, pool=psD, sb=sbD):
            f()
        emit_proj(0, kcs=[5], pr=pr0)
        emit_proj(1, kcs=[5], pr=pr1)
        for t in range(4):
            emit_clsscore(t)
        emit_proj(2)
        for t in range(4, 8):
            emit_clsscore(t)
        emit_proj(3)
        for h in range(6):
            emit_clsacc(h)
        emit_proj(4)
        for h in range(6, 12):
            emit_clsacc(h)
        emit_proj(5)
        emit_proj(6)
        emit_proj(7)
        nc.vector.reciprocal(clsrcp[:], clsps[64:128, :])
        for h in range(H):
            hj, hp = h // 2, 64 * (h % 2)
            nc.vector.tensor_mul(oT[hj][hp:hp + 64, 1024:1025],
                                 clsps[0:64, h:h + 1], clsrcp[:, h:h + 1])
        emit_proj(8)

    ctx.close()


def _build():
    nc = bacc.Bacc(trn_type="TRN2", target_bir_lowering=False)
    with tile.TileContext(nc) as tc:
        _build_body(tc)
    nc.finalize()
    return nc


def _host_tables(xpos_b):
    # cos/sin tables for patch tokens (1..1024), rows = 128 head-pair dims.
    py = xpos_b[1:, 0].astype(np.float64)
    px = xpos_b[1:, 1].astype(np.float64)
    inv = 1.0 / (100.0 ** (np.arange(0, 32, dtype=np.float64)[::2] / 32.0))
    angy = inv[:, None] * py[None, :]
    angx = inv[:, None] * px[None, :]
    c64 = np.concatenate([np.cos(angy), np.cos(angy), np.cos(angx), np.cos(angx)], 0)
    s64 = np.concatenate([np.sin(angy), np.sin(angy), np.sin(angx), np.sin(angx)], 0)
    c128 = np.concatenate([c64, c64], 0)
    s128 = np.concatenate([s64, s64], 0)
    # fold rotate-half signs into the sin table: rows (r%32)<16 negated
    r = np.arange(128)
    s128[(r % 32) < 16] *= -1.0
    bf = ml_dtypes.bfloat16
    return (np.ascontiguousarray(c128.astype(bf)),
            np.ascontiguousarray(s128.astype(bf)))


def kernel(**inputs):
    bf = ml_dtypes.bfloat16
    x = np.asarray(inputs["x"], np.float32)            # [8,1025,768]
    xpos = np.asarray(inputs["xpos"])                  # [8,1025,2]
    w_qkv = np.asarray(inputs["w_qkv"], np.float32)
    w_proj = np.asarray(inputs["w_proj"], np.float32)
    b_proj = np.asarray(inputs["b_proj"], np.float32).reshape(1, DIM)
    num_cls = int(np.asarray(inputs["num_cls"]))
    assert num_cls == 1, f"kernel specialized for num_cls=1, got {num_cls}"

    if "nc" not in _CACHE:
        _CACHE["nc"] = _build()
    nc = _CACHE["nc"]

    wq_bf = np.ascontiguousarray(w_qkv.T.astype(bf))       # [768, 2304]
    wp_bf = np.ascontiguousarray(w_proj.T.astype(bf))      # [768, 768]
    bp = np.ascontiguousarray(b_proj.astype(bf))
    sel = np.zeros((12, DIM), np.float32)
    for h in range(12):
        sel[h, h * 64:(h + 1) * 64] = 1.0
    sel = np.ascontiguousarray(sel.astype(bf))

    in_maps = []
    for b in range(NC):
        c128, s128 = _host_tables(xpos[b])
        xT = np.ascontiguousarray(x[b, 1:, :].T.astype(bf))  # [768, 1024]
        qkv0 = w_qkv @ x[b, 0, :]                            # [2304] cls qkv
        q0, k0, v0 = qkv0[0:DIM], qkv0[DIM:2 * DIM], qkv0[2 * DIM:3 * DIM]
        vcls = np.zeros((1, 12, 128), np.float32)
        for h in range(H):
            vcls[0, h, 0:64] = v0[h * 64:(h + 1) * 64]
            vcls[0, h, 64:128] = 1.0
        # ufix[:, h] = Wq_h^T k0_h  (fixE = ufix^T x = q . k_cls)
        # wfix[:, h] = Wk_h^T q0_h  (psT = x^T wfix = q_cls . k)
        ufix = np.zeros((DIM, 12), np.float32)
        wfix = np.zeros((DIM, 12), np.float32)
        for h in range(H):
            sl = slice(h * 64, (h + 1) * 64)
            ufix[:, h] = w_qkv[sl, :].T @ k0[sl]
            wfix[:, h] = w_qkv[DIM + h * 64:DIM + (h + 1) * 64, :].T @ q0[sl]
        ufix_p = np.zeros((128, 72), np.float32)
        wfix_p = np.zeros((128, 72), np.float32)
        for kc in range(6):
            ufix_p[:, kc * 12:(kc + 1) * 12] = ufix[kc * 128:(kc + 1) * 128]
            wfix_p[:, kc * 12:(kc + 1) * 12] = wfix[kc * 128:(kc + 1) * 128]
        pcls = np.zeros((1, 12), np.float32)
        for h in range(H):
            pcls[0, h] = np.exp(
                float(q0[h * 64:(h + 1) * 64] @ k0[h * 64:(h + 1) * 64])
                * SCALE)
        in_maps.append({
            "xT": xT,
            "wq": wq_bf, "wp": wp_bf,
            "ct": c128, "sst": s128,
            "vcls": np.ascontiguousarray(vcls.reshape(1, 12 * 128).astype(bf)),
            "ufix": np.ascontiguousarray(ufix_p.astype(bf)),
            "wfix": np.ascontiguousarray(wfix_p.astype(bf)),
            "pcls": np.ascontiguousarray(pcls.astype(bf)),
            "sel": sel,
            "bp": bp,
        })
    res = run_bass_kernel_spmd(nc, in_maps, core_ids=list(range(NC)),
                               trace=bool(int(__import__("os").environ.get("BASS_TRACE_KERNEL", "0"))))
    _CACHE["last_result"] = res
    return np.stack([r["out"] for r in res.results], 0)
